# revision 16
# baseline (speedup 1.0000x reference)
"""Trainium2 Bass kernel for nn_CDA_attention (density-modulated attention).

Contract: kernel(**full_inputs) -> full output [8, 256, 64, 64] float32.
Data-parallel over batch: core b computes batch b.

v2: fp8 DoubleRow matmuls (0.5 cycles/row, 256-contraction per instruction)
for q/k/v projections, QK^T and attn@V; exp split between ACT (native Exp ->
fp8e5) and DVE (one-pass Schraudolph: int8 convert of a*s+b bitcast as fp8e5);
per-key softmax scale folded into the fp8 k by the Pool engine; final
normalize+transpose+residual via DMA-transpose + Pool with f32 residual.

Layouts per core (batch b, C=256, N=4096):
  x_parts   [P, CC, N] f32r   (residual + gray)
  x8        [P, CC, N] e4m3   (projection operand; pool convert)
  k8        [P, CC, N] e4m3   = (Wk16 x + 16 kb) * skv / 16  (skv = C^-.5/tau)
  q8/it     [P, CC, NQT] e4m3 = Wq16 x + 16 qb   (16x weight scale cancels
                                the 1/16 in k8's fold inside the QK product)
  v8        [P, NKC, C+1] e4m3 = ((Wout Wv) x)/16 transposed, ones column
  exp_sb    [P, NKC, NQT] e5m2 = exp(scores)
  pos       [P, NSUB, 512] f32 psum: attn@V + rowsums (col 256)
  osbd -> dma-transpose -> osbT -> *rcp_bcast + x16b -> out
"""

import os
import sys

sys.path.insert(0, "/opt/trn_rl_repo")

from contextlib import ExitStack

import numpy as np
import ml_dtypes

import concourse.bass as bass
import concourse.mybir as mybir
import concourse.tile as tile
from concourse import bacc, bass_utils
from concourse.masks import make_identity

B, C, HH, WW = 8, 256, 64, 64
N = HH * WW          # 4096
P = 128
CC = C // P          # 2 channel chunks
NQT = 512            # query tile
NQ_TILES = N // NQT  # 8
NKC = N // P         # 32 key chunks
NPAIR = NKC // 2     # 16 key-chunk pairs
NSUB = NQT // P      # 4 query sub-tiles

f32 = mybir.dt.float32
f32r = mybir.dt.float32r
bf16 = mybir.dt.bfloat16
e4 = mybir.dt.float8e4
e5 = mybir.dt.float8e5
i8 = mybir.dt.int8
AF = mybir.ActivationFunctionType
ALU = mybir.AluOpType
PM = mybir.MatmulPerfMode
DR = PM.DoubleRow

LOG2E = 1.4426950408889634
# scores arrive in PSUM as 16x the true scaled score (q carries the 16x host
# weight scale; k's fold divides its own 16x out) -> both exp paths rescale
SCORE_DESCALE = 1.0 / 16.0
SCH_A = 4.0 * LOG2E * SCORE_DESCALE  # e5m2 exponent-domain scale
SCH_B = 60.0          # 15(bias)*4; constant factor cancels in softmax

# which key-chunk pairs the DVE (Schraudolph) path handles; ACT takes the rest
DVE_JJ = frozenset({1, 3, 5, 7, 9, 11, 13})

# tap order for 3x3 convs: center first so the first tap writes the full tile
TAPS = [(1, 1)] + [(ky, kx) for ky in range(3) for kx in range(3) if (ky, kx) != (1, 1)]


def _make_row_shifted(nc, pool, src, name):
    """{dy: AP} of row-shifted copies of src ([64, ...] SBUF tile) via DMA."""
    shape = list(src.shape)
    p1 = pool.tile(shape, src.dtype, name=f"{name}_p1")
    m1 = pool.tile(shape, src.dtype, name=f"{name}_m1")
    nc.gpsimd.memset(p1[:], 0.0)
    nc.gpsimd.memset(m1[:], 0.0)
    nc.sync.dma_start(p1[0:63], src[1:64])
    nc.sync.dma_start(m1[1:64], src[0:63])
    return {0: src, 1: p1, -1: m1}


def build_kernel_body(tc, ctx, d):
    nc = tc.nc
    x_d, wqk_d, wvo_d = d["x"], d["wqk8"], d["wvo8"]
    qkbq_d, qkbk_d, bfin_d = d["qkb16q"], d["qkb16k"], d["bfin"]
    w1b_d, w1bias_d, w2b_d, w2bias_d = d["w1b"], d["w1bias"], d["w2b"], d["w2bias"]
    out_d, scr1, scr2, scr3 = d["out"], d["scr1"], d["scr2"], d["scr3"]

    const = ctx.enter_context(tc.tile_pool(name="const", bufs=1))
    big = ctx.enter_context(tc.tile_pool(name="big", bufs=1))
    ps_pool = ctx.enter_context(tc.tile_pool(name="ps", bufs=2, space="PSUM"))
    pos_pool = ctx.enter_context(tc.tile_pool(name="pos", bufs=1, space="PSUM"))
    qt_pool = ctx.enter_context(tc.tile_pool(name="qt", bufs=2))
    osbd_pool = ctx.enter_context(tc.tile_pool(name="osbd", bufs=2))
    osbT_pool = ctx.enter_context(tc.tile_pool(name="osbT", bufs=2))
    fin_pool = ctx.enter_context(tc.tile_pool(name="fin", bufs=2))
    rcp_pool = ctx.enter_context(tc.tile_pool(name="rcp", bufs=2))
    rcpb_pool = ctx.enter_context(tc.tile_pool(name="rcpb", bufs=2))
    grow_pool = ctx.enter_context(tc.tile_pool(name="grow", bufs=2))

    # ---- persistent SBUF tiles ----
    XQ = N // 4
    x_parts = [big.tile([P, CC, XQ], f32r, name=f"xp{t}") for t in range(4)]

    def x_slice(ci, start, size):
        t = start // XQ
        o = start - t * XQ
        return x_parts[t][:, ci, o:o + size]

    x8 = [big.tile([P, CC, XQ], e4, name=f"x8_{t}") for t in range(4)]

    def x8_slice(start, size):
        t = start // XQ
        o = start - t * XQ
        return x8[t][:, :, o:o + size]

    x16b = [big.tile([P, CC, XQ], f32, name=f"x16b_{t}") for t in range(4)]

    def x16b_slice(start, size):
        t = start // XQ
        o = start - t * XQ
        return x16b[t][:, :, o:o + size]

    k16 = big.tile([P, CC, N], bf16)
    k8 = big.tile([P, CC, N], e4)
    v8 = big.tile([P, NKC, C + 1], e4)
    exp_a = big.tile([P, NKC, NQT], e5)
    exp_b = big.tile([P, NKC, NQT], e5)
    repl = big.tile([P, 1, N], f32)
    wqk_sb = const.tile([P, CC, 2 * C], e4)
    wvo_sb = const.tile([P, CC, C], e4)
    qkbq_sb = const.tile([P, CC], f32)
    qkbk_sb = const.tile([P, CC], f32)
    bfin_sb = const.tile([P, CC], f32)
    w1b_sb = const.tile([64, 9, 8, 1], bf16)
    w1bias_sb = const.tile([64, 8, 1], bf16)
    w2b_sb = const.tile([64, 9, 8, 1], bf16)
    w2bias_sb = const.tile([64, 1], f32)
    ones_sb = const.tile([P, 1], f32r)
    ident16 = const.tile([P, P], bf16)
    gray_img = const.tile([64, 64], f32)
    lap_t = const.tile([64, 64], f32)
    abs_t = const.tile([64, 1, 64], bf16)
    h1_t = const.tile([64, 8, 64], bf16)
    h1r_t = const.tile([64, 8, 64], bf16)
    cacc_t = const.tile([64, 8, 64], bf16)
    ctmp_t = const.tile([64, 8, 64], bf16)
    dl_t = const.tile([64, 64], bf16)
    sig_t = const.tile([64, 64], f32)
    skv_t = const.tile([64, 64], f32)

    # ---- input DMAs ----
    for ci in range(CC):
        nc.sync.dma_start(wqk_sb[:, ci, :], wqk_d[ci * P:(ci + 1) * P, :])
        nc.sync.dma_start(wvo_sb[:, ci, :], wvo_d[ci * P:(ci + 1) * P, :])
    nc.sync.dma_start(ones_sb[:, :], d["ones"][:, :])
    nc.sync.dma_start(qkbq_sb[:, :], qkbq_d[:, :])
    nc.sync.dma_start(qkbk_sb[:, :], qkbk_d[:, :])
    nc.sync.dma_start(bfin_sb[:, :], bfin_d[:, :])
    nc.sync.dma_start(w1b_sb[:, :, :, 0], w1b_d.rearrange("p (t o) -> p t o", o=8))
    nc.sync.dma_start(w1bias_sb[:, :, 0], w1bias_d[:, :])
    nc.sync.dma_start(w2b_sb[:, :, :, 0], w2b_d.rearrange("p (t o) -> p t o", o=8))
    nc.sync.dma_start(w2bias_sb[:, :], w2bias_d[:, :])
    for t in range(4):
        for ci in range(CC):
            for h in range(2):
                nc.sync.dma_start(
                    x_parts[t][:, ci, h * (XQ // 2):(h + 1) * (XQ // 2)],
                    x_d[ci * P:(ci + 1) * P,
                        t * XQ + h * (XQ // 2):t * XQ + (h + 1) * (XQ // 2)])

    nc.gpsimd.memset(v8[:, :, C:C + 1], 1.0)   # ones columns -> row sums
    make_identity(nc, ident16)

    # fp8 x copy (projection operand), per quarter as DMA lands; split between
    # ACT and DVE (both idle during the input DMA window; Pool is too slow)
    for t in range(4):
        if t < 2:
            nc.scalar.activation(
                x8[t][:, :, :], x_parts[t][:, :, :].bitcast(f32), AF.Copy)
        else:
            nc.vector.tensor_copy(x8[t][:, :, :], x_parts[t][:, :, :].bitcast(f32))


    # ---- gray = mean_c x (PE, f32r) ----
    scr1_2d = scr1.rearrange("(a b) -> a b", a=1)
    for nt in range(NQ_TILES):
        pg = ps_pool.tile([1, NQT], f32, tag="ps")
        for ci in range(CC):
            nc.tensor.matmul(
                pg[:, :], ones_sb[:, :], x_slice(ci, nt * NQT, NQT),
                start=(ci == 0), stop=(ci == CC - 1))
        grow = grow_pool.tile([1, NQT], f32)
        nc.vector.tensor_copy(grow[:, :], pg[:, :])
        nc.sync.dma_start(scr1_2d[:, nt * NQT:(nt + 1) * NQT], grow[:, :])

    nc.sync.dma_start(gray_img[:, :], scr1.rearrange("(h w) -> h w", w=64))
    g_p1 = const.tile([64, 64], f32)
    g_m1 = const.tile([64, 64], f32)
    nc.gpsimd.memset(g_p1[:], 0.0)
    nc.gpsimd.memset(g_m1[:], 0.0)
    sh = scr1.rearrange("(h w) -> h w", w=64)
    nc.sync.dma_start(g_p1[0:63, :], sh[1:64, :])
    nc.sync.dma_start(g_m1[1:64, :], sh[0:63, :])
    gvar = {0: gray_img, 1: g_p1, -1: g_m1}

    # ---- density part A: Laplacian (DVE) + abs (ACT), emitted BEFORE the
    # projections so these small ops sit ahead of the eviction streams in the
    # in-order DVE/ACT queues ----
    nc.vector.tensor_scalar(
        out=lap_t[:, :], in0=gray_img[:, :], scalar1=4.0, scalar2=None, op0=ALU.mult)
    for dy in (1, -1):
        nc.vector.scalar_tensor_tensor(
            out=lap_t[:, :], in0=gvar[dy][:, :], scalar=-1.0, in1=lap_t[:, :],
            op0=ALU.mult, op1=ALU.add)
    for dx in (1, -1):
        c0, c1 = max(0, -dx), WW - max(0, dx)
        dst = lap_t[:, c0:c1]
        nc.vector.scalar_tensor_tensor(
            out=dst, in0=gray_img[:, c0 + dx:c1 + dx], scalar=-1.0, in1=dst,
            op0=ALU.mult, op1=ALU.add)
    nc.scalar.activation(abs_t[:, 0, :], lap_t[:, :], AF.Abs)

    # ---- k projection (fp8 DoubleRow) + DVE evict to bf16 ----
    for nt in range(NQ_TILES):
        pq = ps_pool.tile([P, CC, NQT], f32, tag="ps")
        for m in range(CC):                # k output-channel chunks
            nc.tensor.matmul(
                pq[:, m, :],
                wqk_sb[:, :, (2 + m) * P:(3 + m) * P],
                x8_slice(nt * NQT, NQT),
                start=True, stop=True, perf_mode=DR)
        for m in range(CC):
            nc.scalar.activation(
                k16[:, m, nt * NQT:(nt + 1) * NQT], pq[:, m, :],
                AF.Identity, bias=qkbk_sb[:, m:m + 1])

    # ---- vproj = ((Wout Wv)/16) x transposed (fp8 DR) + ACT evict ----
    for jj in range(NPAIR):
        pv = ps_pool.tile([P, 2, NQT], f32, tag="ps")
        for u in range(2):
            j = 2 * jj + u
            nc.tensor.matmul(
                pv[:, u, 0:C],
                x8_slice(j * P, P),
                wvo_sb[:, :, :],
                start=True, stop=True, perf_mode=DR)
        nc.scalar.activation(
            v8[:, 2 * jj:2 * jj + 2, 0:C], pv[:, :, 0:C], AF.Copy,
            scale=1.0 / 16.0)

    # ---- density part B (gray -> skv), convs on Pool ----
    avar = _make_row_shifted(nc, const, abs_t, "abs")

    def conv_taps(out_t, in_var, wpat):
        for i, (ky, kx) in enumerate(TAPS):
            dy, dx = ky - 1, kx - 1
            c0, c1 = max(0, -dx), WW - max(0, dx)
            L = c1 - c0
            src = in_var(dy, slice(c0 + dx, c1 + dx))
            w = wpat[:, ky * 3 + kx, :, :].broadcast_to([64, 8, L])
            if i == 0:
                nc.vector.tensor_mul(out_t[:, :, :], src, w)
            else:
                nc.vector.tensor_mul(ctmp_t[:, :, 0:L], src, w)
                nc.vector.tensor_add(
                    out_t[:, :, c0:c1], out_t[:, :, c0:c1], ctmp_t[:, :, 0:L])

    conv_taps(
        h1_t,
        lambda dy, cs: avar[dy][:, :, cs].broadcast_to(
            [64, 8, cs.stop - cs.start]),
        w1b_sb)
    nc.vector.tensor_add(
        h1_t[:, :, :], h1_t[:, :, :], w1bias_sb.broadcast_to([64, 8, WW]))
    nc.vector.tensor_scalar(
        out=h1r_t[:, :, :], in0=h1_t[:, :, :], scalar1=0.0, scalar2=None,
        op0=ALU.max)

    hvar = _make_row_shifted(nc, const, h1r_t, "h1r")
    conv_taps(cacc_t, lambda dy, cs: hvar[dy][:, :, cs], w2b_sb)
    nc.vector.tensor_add(cacc_t[:, 0:4, :], cacc_t[:, 0:4, :], cacc_t[:, 4:8, :])
    nc.vector.tensor_add(cacc_t[:, 0:2, :], cacc_t[:, 0:2, :], cacc_t[:, 2:4, :])
    nc.vector.tensor_add(dl_t[:, :], cacc_t[:, 0, :], cacc_t[:, 1, :])
    nc.scalar.activation(sig_t[:, :], dl_t[:, :], AF.Sigmoid, bias=w2bias_sb[:, 0:1])
    # skv = (C^-0.5/16) / (3 - 2*sigmoid); /16 compensates the 16x k weights
    nc.scalar.activation(dl_t[:, :], sig_t[:, :], AF.Copy, bias=3.0, scale=-2.0)
    nc.vector.reciprocal(sig_t[:, :], dl_t[:, :])
    nc.vector.tensor_scalar(
        out=skv_t[:, :], in0=sig_t[:, :], scalar1=float(C) ** -0.5 / 16.0,
        scalar2=None, op0=ALU.mult)
    nc.sync.dma_start(scr2.rearrange("(h w) -> h w", w=64), skv_t[:, :])

    # skv broadcast [P, N] (chunked so the k8 fold can start early) and the
    # pool fold k8 = k16 * skv
    scr2_1 = scr2.rearrange("(a b) -> a b", a=1)
    for nt in range(NQ_TILES):
        sl = slice(nt * NQT, (nt + 1) * NQT)
        nc.sync.dma_start(
            repl[:, 0, sl], scr2_1[0:1, sl].broadcast_to([P, NQT]))
        nc.vector.tensor_mul(
            k8[:, :, sl], k16[:, :, sl],
            repl[:, :, sl].broadcast_to([P, CC, NQT]))


    # x16b = x + bfin (residual with the final bias folded); ACT, emitted
    # here so the upfront eviction streams aren't delayed behind it
    for t in range(4):
        for ci in range(CC):
            nc.scalar.activation(
                x16b[t][:, ci, :], x_parts[t][:, ci, :].bitcast(f32),
                AF.Identity, bias=bfin_sb[:, ci:ci + 1])

    # ---- attention loop ----
    # scr3 linear index it*512 + s*128 + p must receive rcp[p, s]: view the
    # DRAM as [it, p, s] for the write, [it, q] (q = s*128+p) for the read
    scr3_3 = scr3.rearrange("(it s p) -> it p s", s=NSUB, p=P)
    scr3_2 = scr3.rearrange("(it q) -> it q", q=NQT)
    for it in range(NQ_TILES):
        nq0 = it * NQT
        exp_sb = exp_a if it % 2 == 0 else exp_b

        # q tile projection (fp8 DR), DVE evict straight to e4 with bias
        q8_t = qt_pool.tile([P, CC, NQT], e4)
        pq = ps_pool.tile([P, CC, NQT], f32, tag="ps")
        for mm in range(CC):
            nc.tensor.matmul(
                pq[:, mm, :],
                wqk_sb[:, :, mm * P:(mm + 1) * P],
                x8_slice(nq0, NQT),
                start=True, stop=True, perf_mode=DR)
        for mm in range(CC):
            nc.vector.tensor_scalar(
                out=q8_t[:, mm, :], in0=pq[:, mm, :],
                scalar1=qkbq_sb[:, mm:mm + 1], scalar2=None, op0=ALU.add)

        pos = pos_pool.tile([P, NSUB, NQT], f32)

        def attnv_pair(jj):
            for s in range(NSUB):
                nc.tensor.matmul(
                    pos[:, s, 0:C + 1],
                    exp_sb[:, 2 * jj:2 * jj + 2, s * P:(s + 1) * P],
                    v8[:, 2 * jj:2 * jj + 2, :],
                    start=(jj == 0), stop=(jj == NPAIR - 1), perf_mode=DR)

        for jj in range(NPAIR):
            ps2 = ps_pool.tile([P, 2, NQT], f32, tag="ps")
            for u in range(2):
                j = 2 * jj + u
                nc.tensor.matmul(
                    ps2[:, u, :],
                    k8[:, :, j * P:(j + 1) * P],
                    q8_t[:, :, :],
                    start=True, stop=True, perf_mode=DR)
            dst = exp_sb[:, 2 * jj:2 * jj + 2, :]
            if jj in DVE_JJ:
                nc.vector.tensor_scalar(
                    out=dst.bitcast(i8), in0=ps2[:, :, :],
                    scalar1=SCH_A, scalar2=SCH_B, op0=ALU.mult, op1=ALU.add)
            else:
                nc.scalar.activation(dst, ps2[:, :, :], AF.Exp, scale=SCORE_DESCALE)
            if jj >= 1:
                attnv_pair(jj - 1)
        attnv_pair(NPAIR - 1)

        # rowsum reciprocals -> DRAM roundtrip -> broadcast over partitions
        # (dst AP is the transposed DRAM view so SBUF keeps partition-first)
        rcp = rcp_pool.tile([P, NSUB], f32)
        nc.vector.reciprocal(rcp[:, :], pos[:, :, C])
        nc.sync.dma_start(scr3_3[it, :, :], rcp[:, :])
        rcpb = rcpb_pool.tile([P, 1, NQT], f32)
        nc.sync.dma_start(
            rcpb[:, 0, :], scr3_2[it:it + 1, :].broadcast_to([P, NQT]))

        # ACT evicts attn@V block to bf16 [q, (s), c]
        osbd = osbd_pool.tile([P, NSUB, C], bf16)
        nc.scalar.activation(osbd[:, :, :], pos[:, :, 0:C], AF.Copy)

        # PE transposes each [128q, 128c] block into a shared 2-bank PSUM
        # tile (slot ci*4+s); start once per bank, accumulate onto the
        # pending-zero region for the other slots
        pt = ps_pool.tile([P, CC * NSUB, 2 * P], bf16, tag="ps")
        for ci in range(CC):
            for s in range(NSUB):
                nc.tensor.matmul(
                    pt[:, ci * NSUB + s, 0:P],
                    osbd[:, s, ci * P:(ci + 1) * P], ident16[:, :],
                    is_transpose=True, start=(s == 0), stop=(s == NSUB - 1))

        # DVE evicts pt (freeing the ps buffer), pool normalizes + residual
        t0 = osbT_pool.tile([P, CC, NQT], bf16)
        nc.vector.tensor_copy(t0[:, :, :], pt[:, :, 0:P])
        fin = fin_pool.tile([P, CC, NQT], f32)
        nc.gpsimd.tensor_mul(
            fin[:, :, :], t0[:, :, :],
            rcpb[:, :, :].broadcast_to([P, CC, NQT]))
        nc.gpsimd.tensor_add(
            fin[:, :, :], fin[:, :, :], x16b_slice(nq0, NQT))
        for ci in range(CC):
            for h in range(2):
                nc.sync.dma_start(
                    out_d[ci * P:(ci + 1) * P,
                          nq0 + h * (NQT // 2):nq0 + (h + 1) * (NQT // 2)],
                    fin[:, ci, h * (NQT // 2):(h + 1) * (NQT // 2)])


def build_nc():
    nc = bacc.Bacc("TRN2", target_bir_lowering=False, debug=False)
    d = {}

    def inp(name, shape, dt=f32):
        d[name] = nc.dram_tensor(name, shape, dt, kind="ExternalInput").ap()

    inp("x", (C, N), f32r)
    inp("wqk8", (C, 2 * C), e4)
    inp("wvo8", (C, C), e4)
    inp("ones", (P, 1), f32r)
    inp("qkb16q", (P, CC))
    inp("qkb16k", (P, CC))
    inp("bfin", (P, CC))
    inp("w1b", (64, 72), bf16)
    inp("w1bias", (64, 8), bf16)
    inp("w2b", (64, 72), bf16)
    inp("w2bias", (64, 1))
    d["out"] = nc.dram_tensor("out", (C, N), f32, kind="ExternalOutput").ap()
    d["scr1"] = nc.dram_tensor("scr1", (N,), f32, kind="Internal").ap()
    d["scr2"] = nc.dram_tensor("scr2", (N,), f32, kind="Internal").ap()
    d["scr3"] = nc.dram_tensor("scr3", (N,), f32, kind="Internal").ap()

    with tile.TileContext(nc) as tc, ExitStack() as ctx:
        build_kernel_body(tc, ctx, d)
    nc.compile()
    return nc


def host_inputs(x, qkv_w, qkv_b, out_w, out_b, d1_w, d1_b, d2_w, d2_b):
    f = np.float32
    x = np.asarray(x, f)
    wq = np.asarray(qkv_w, f)[:, :, 0, 0]          # [768, 256]
    qkv_b = np.asarray(qkv_b, f)
    wout = np.asarray(out_w, f)[:, :, 0, 0]        # [256, 256]
    out_b = np.asarray(out_b, f)
    e4m3 = ml_dtypes.float8_e4m3
    shared = {
        "wqk8": np.ascontiguousarray(16.0 * wq[0:2 * C].T).astype(e4m3),
        "wvo8": np.ascontiguousarray(
            16.0 * (wout @ wq[2 * C:3 * C]).T).astype(e4m3),
        "qkb16q": np.ascontiguousarray(
            16.0 * qkv_b[0:C].reshape(CC, P).T.astype(f)),
        "qkb16k": np.ascontiguousarray(
            16.0 * qkv_b[C:2 * C].reshape(CC, P).T.astype(f)),
        "bfin": np.ascontiguousarray(
            (wout @ qkv_b[2 * C:3 * C] + out_b).reshape(CC, P).T.astype(f)),
        "w1b": np.tile(
            np.ascontiguousarray(np.asarray(d1_w, f).reshape(8, 9).T).reshape(1, 72),
            (64, 1)).astype(f),
        "w1bias": np.tile(np.asarray(d1_b, f).reshape(1, 8), (64, 1)).astype(f),
        "w2b": np.tile(
            np.ascontiguousarray(np.asarray(d2_w, f).reshape(8, 9).T).reshape(1, 72),
            (64, 1)).astype(f),
        "w2bias": np.tile(np.asarray(d2_b, f).reshape(1, 1), (64, 1)).astype(f),
        "ones": np.full((P, 1), 1.0 / C, f),
    }
    for kk in ("w1b", "w1bias", "w2b"):
        shared[kk] = shared[kk].astype(ml_dtypes.bfloat16)
    xs = x.reshape(B, C, N)
    return [dict(x=np.ascontiguousarray(xs[b]), **shared) for b in range(B)]


_NC_CACHE = {}


def _get_nc():
    if "nc" not in _NC_CACHE:
        _NC_CACHE["nc"] = build_nc()
    return _NC_CACHE["nc"]


def kernel(x, qkv_w, qkv_b, out_w, out_b, d1_w, d1_b, d2_w, d2_b):
    in_maps = host_inputs(x, qkv_w, qkv_b, out_w, out_b, d1_w, d1_b, d2_w, d2_b)
    nc = _get_nc()
    trace = bool(int(os.environ.get("KERNEL_TRACE", "0")))
    res = bass_utils.run_bass_kernel_spmd(
        nc, in_maps, core_ids=list(range(B)), trace=trace)
    _NC_CACHE["last_results"] = res
    out = np.stack([res.results[b]["out"] for b in range(B)])
    return np.ascontiguousarray(out.reshape(B, C, HH, WW).astype(np.float32))


# revision 17
# speedup vs baseline: 1.0727x; 1.0727x over previous
"""Trainium2 Bass kernel for nn_CDA_attention (density-modulated attention).

Contract: kernel(**full_inputs) -> full output [8, 256, 64, 64] float32.
Data-parallel over batch: core b computes batch b.

v2: fp8 DoubleRow matmuls (0.5 cycles/row, 256-contraction per instruction)
for q/k/v projections, QK^T and attn@V; exp split between ACT (native Exp ->
fp8e5) and DVE (one-pass Schraudolph: int8 convert of a*s+b bitcast as fp8e5);
per-key softmax scale folded into the fp8 k by the Pool engine; final
normalize+transpose+residual via DMA-transpose + Pool with f32 residual.

Layouts per core (batch b, C=256, N=4096):
  x_parts   [P, CC, N] f32r   (residual + gray)
  x8        [P, CC, N] e4m3   (projection operand; pool convert)
  k8        [P, CC, N] e4m3   = (Wk16 x + 16 kb) * skv / 16  (skv = C^-.5/tau)
  q8/it     [P, CC, NQT] e4m3 = Wq16 x + 16 qb   (16x weight scale cancels
                                the 1/16 in k8's fold inside the QK product)
  v8        [P, NKC, C+1] e4m3 = ((Wout Wv) x)/16 transposed, ones column
  exp_sb    [P, NKC, NQT] e5m2 = exp(scores)
  pos       [P, NSUB, 512] f32 psum: attn@V + rowsums (col 256)
  osbd -> dma-transpose -> osbT -> *rcp_bcast + x16b -> out
"""

import os
import sys

sys.path.insert(0, "/opt/trn_rl_repo")

from contextlib import ExitStack

import numpy as np
import ml_dtypes

import concourse.bass as bass
import concourse.mybir as mybir
import concourse.tile as tile
from concourse import bacc, bass_utils
from concourse.masks import make_identity

B, C, HH, WW = 8, 256, 64, 64
N = HH * WW          # 4096
P = 128
CC = C // P          # 2 channel chunks
NQT = 512            # query tile
NQ_TILES = N // NQT  # 8
NKC = N // P         # 32 key chunks
NPAIR = NKC // 2     # 16 key-chunk pairs
NSUB = NQT // P      # 4 query sub-tiles

f32 = mybir.dt.float32
f32r = mybir.dt.float32r
bf16 = mybir.dt.bfloat16
e4 = mybir.dt.float8e4
e5 = mybir.dt.float8e5
i8 = mybir.dt.int8
AF = mybir.ActivationFunctionType
ALU = mybir.AluOpType
PM = mybir.MatmulPerfMode
DR = PM.DoubleRow

LOG2E = 1.4426950408889634
# scores arrive in PSUM as 16x the true scaled score (q carries the 16x host
# weight scale; k's fold divides its own 16x out) -> both exp paths rescale
SCORE_DESCALE = 1.0 / 16.0
SCH_A = 4.0 * LOG2E * SCORE_DESCALE  # e5m2 exponent-domain scale
SCH_B = 60.0          # 15(bias)*4; constant factor cancels in softmax

# which key-chunk pairs the DVE (Schraudolph) path handles; ACT takes the rest
DVE_JJ = frozenset({1, 3, 5, 7, 9, 11})

# tap order for 3x3 convs: center first so the first tap writes the full tile
TAPS = [(1, 1)] + [(ky, kx) for ky in range(3) for kx in range(3) if (ky, kx) != (1, 1)]


def _make_row_shifted(nc, pool, src, name):
    """{dy: AP} of row-shifted copies of src ([64, ...] SBUF tile) via DMA."""
    shape = list(src.shape)
    p1 = pool.tile(shape, src.dtype, name=f"{name}_p1")
    m1 = pool.tile(shape, src.dtype, name=f"{name}_m1")
    nc.gpsimd.memset(p1[:], 0.0)
    nc.gpsimd.memset(m1[:], 0.0)
    nc.sync.dma_start(p1[0:63], src[1:64])
    nc.sync.dma_start(m1[1:64], src[0:63])
    return {0: src, 1: p1, -1: m1}


def build_kernel_body(tc, ctx, d):
    nc = tc.nc
    x_d, wqk_d, wvo_d = d["x"], d["wqk8"], d["wvo8"]
    qkbq_d, qkbk_d, bfin_d = d["qkb16q"], d["qkb16k"], d["bfin"]
    w1b_d, w1bias_d, w2b_d, w2bias_d = d["w1b"], d["w1bias"], d["w2b"], d["w2bias"]
    out_d, scr1, scr2, scr3 = d["out"], d["scr1"], d["scr2"], d["scr3"]

    const = ctx.enter_context(tc.tile_pool(name="const", bufs=1))
    big = ctx.enter_context(tc.tile_pool(name="big", bufs=1))
    ps_pool = ctx.enter_context(tc.tile_pool(name="ps", bufs=2, space="PSUM"))
    pos_pool = ctx.enter_context(tc.tile_pool(name="pos", bufs=1, space="PSUM"))
    qt_pool = ctx.enter_context(tc.tile_pool(name="qt", bufs=2))
    osbd_pool = ctx.enter_context(tc.tile_pool(name="osbd", bufs=2))
    osbT_pool = ctx.enter_context(tc.tile_pool(name="osbT", bufs=2))
    fin_pool = ctx.enter_context(tc.tile_pool(name="fin", bufs=2))
    rcp_pool = ctx.enter_context(tc.tile_pool(name="rcp", bufs=2))
    rcpb_pool = ctx.enter_context(tc.tile_pool(name="rcpb", bufs=2))
    grow_pool = ctx.enter_context(tc.tile_pool(name="grow", bufs=2))

    # ---- persistent SBUF tiles ----
    XQ = N // 4
    x_parts = [big.tile([P, CC, XQ], f32r, name=f"xp{t}") for t in range(4)]

    def x_slice(ci, start, size):
        t = start // XQ
        o = start - t * XQ
        return x_parts[t][:, ci, o:o + size]

    x8 = [big.tile([P, CC, XQ], e4, name=f"x8_{t}") for t in range(4)]

    def x8_slice(start, size):
        t = start // XQ
        o = start - t * XQ
        return x8[t][:, :, o:o + size]

    x16b = [big.tile([P, CC, XQ], f32, name=f"x16b_{t}") for t in range(4)]

    def x16b_slice(start, size):
        t = start // XQ
        o = start - t * XQ
        return x16b[t][:, :, o:o + size]

    k16 = big.tile([P, CC, N], bf16)
    k8 = big.tile([P, CC, N], e4)
    v8 = big.tile([P, NKC, C + 1], e4)
    exp_a = big.tile([P, NKC, NQT], e5)
    exp_b = big.tile([P, NKC, NQT], e5)
    repl = big.tile([P, 1, N], f32)
    wqk_sb = const.tile([P, CC, 2 * C], e4)
    wvo_sb = const.tile([P, CC, C], e4)
    qkbq_sb = const.tile([P, CC], f32)
    qkbk_sb = const.tile([P, CC], f32)
    bfin_sb = const.tile([P, CC], f32)
    w1b_sb = const.tile([64, 9, 8, 1], bf16)
    w1bias_sb = const.tile([64, 8, 1], bf16)
    w2b_sb = const.tile([64, 9, 8, 1], bf16)
    w2bias_sb = const.tile([64, 1], f32)
    ones_sb = const.tile([P, 1], f32r)
    ident16 = const.tile([P, P], bf16)
    gray_img = const.tile([64, 64], f32)
    lap_t = const.tile([64, 64], f32)
    abs_t = const.tile([64, 1, 64], bf16)
    h1_t = const.tile([64, 8, 64], bf16)
    h1r_t = const.tile([64, 8, 64], bf16)
    cacc_t = const.tile([64, 8, 64], bf16)
    ctmp_t = const.tile([64, 8, 64], bf16)
    dl_t = const.tile([64, 64], bf16)
    sig_t = const.tile([64, 64], f32)
    skv_t = const.tile([64, 64], f32)

    # ---- input DMAs ----
    for ci in range(CC):
        nc.sync.dma_start(wqk_sb[:, ci, :], wqk_d[ci * P:(ci + 1) * P, :])
        nc.sync.dma_start(wvo_sb[:, ci, :], wvo_d[ci * P:(ci + 1) * P, :])
    nc.sync.dma_start(ones_sb[:, :], d["ones"][:, :])
    nc.sync.dma_start(qkbq_sb[:, :], qkbq_d[:, :])
    nc.sync.dma_start(qkbk_sb[:, :], qkbk_d[:, :])
    nc.sync.dma_start(bfin_sb[:, :], bfin_d[:, :])
    nc.sync.dma_start(w1b_sb[:, :, :, 0], w1b_d.rearrange("p (t o) -> p t o", o=8))
    nc.sync.dma_start(w1bias_sb[:, :, 0], w1bias_d[:, :])
    nc.sync.dma_start(w2b_sb[:, :, :, 0], w2b_d.rearrange("p (t o) -> p t o", o=8))
    nc.sync.dma_start(w2bias_sb[:, :], w2bias_d[:, :])
    for t in range(4):
        for ci in range(CC):
            for h in range(2):
                nc.sync.dma_start(
                    x_parts[t][:, ci, h * (XQ // 2):(h + 1) * (XQ // 2)],
                    x_d[ci * P:(ci + 1) * P,
                        t * XQ + h * (XQ // 2):t * XQ + (h + 1) * (XQ // 2)])

    nc.gpsimd.memset(v8[:, :, C:C + 1], 1.0)   # ones columns -> row sums
    make_identity(nc, ident16)

    # fp8 x copy (projection operand), per quarter as DMA lands; split between
    # ACT and DVE (both idle during the input DMA window; Pool is too slow)
    for t in range(4):
        if t < 2:
            nc.scalar.activation(
                x8[t][:, :, :], x_parts[t][:, :, :].bitcast(f32), AF.Copy)
        else:
            nc.vector.tensor_copy(x8[t][:, :, :], x_parts[t][:, :, :].bitcast(f32))


    # ---- gray = mean_c x (PE, f32r) ----
    scr1_2d = scr1.rearrange("(a b) -> a b", a=1)
    for nt in range(NQ_TILES):
        pg = ps_pool.tile([1, NQT], f32, tag="ps")
        for ci in range(CC):
            nc.tensor.matmul(
                pg[:, :], ones_sb[:, :], x_slice(ci, nt * NQT, NQT),
                start=(ci == 0), stop=(ci == CC - 1))
        grow = grow_pool.tile([1, NQT], f32)
        nc.vector.tensor_copy(grow[:, :], pg[:, :])
        nc.sync.dma_start(scr1_2d[:, nt * NQT:(nt + 1) * NQT], grow[:, :])

    nc.sync.dma_start(gray_img[:, :], scr1.rearrange("(h w) -> h w", w=64))
    g_p1 = const.tile([64, 64], f32)
    g_m1 = const.tile([64, 64], f32)
    nc.gpsimd.memset(g_p1[:], 0.0)
    nc.gpsimd.memset(g_m1[:], 0.0)
    sh = scr1.rearrange("(h w) -> h w", w=64)
    nc.sync.dma_start(g_p1[0:63, :], sh[1:64, :])
    nc.sync.dma_start(g_m1[1:64, :], sh[0:63, :])
    gvar = {0: gray_img, 1: g_p1, -1: g_m1}

    # ---- density part A: Laplacian (DVE) + abs (ACT), emitted BEFORE the
    # projections so these small ops sit ahead of the eviction streams in the
    # in-order DVE/ACT queues ----
    nc.vector.tensor_scalar(
        out=lap_t[:, :], in0=gray_img[:, :], scalar1=4.0, scalar2=None, op0=ALU.mult)
    for dy in (1, -1):
        nc.vector.scalar_tensor_tensor(
            out=lap_t[:, :], in0=gvar[dy][:, :], scalar=-1.0, in1=lap_t[:, :],
            op0=ALU.mult, op1=ALU.add)
    for dx in (1, -1):
        c0, c1 = max(0, -dx), WW - max(0, dx)
        dst = lap_t[:, c0:c1]
        nc.vector.scalar_tensor_tensor(
            out=dst, in0=gray_img[:, c0 + dx:c1 + dx], scalar=-1.0, in1=dst,
            op0=ALU.mult, op1=ALU.add)
    nc.scalar.activation(abs_t[:, 0, :], lap_t[:, :], AF.Abs)

    # ---- k projection (fp8 DoubleRow) + DVE evict to bf16 ----
    for nt in range(NQ_TILES):
        pq = ps_pool.tile([P, CC, NQT], f32, tag="ps")
        for m in range(CC):                # k output-channel chunks
            nc.tensor.matmul(
                pq[:, m, :],
                wqk_sb[:, :, (2 + m) * P:(3 + m) * P],
                x8_slice(nt * NQT, NQT),
                start=True, stop=True, perf_mode=DR)
        for m in range(CC):
            nc.scalar.activation(
                k16[:, m, nt * NQT:(nt + 1) * NQT], pq[:, m, :],
                AF.Identity, bias=qkbk_sb[:, m:m + 1])

    # ---- vproj = ((Wout Wv)/16) x transposed (fp8 DR) + ACT evict ----
    for jj in range(NPAIR):
        pv = ps_pool.tile([P, 2, NQT], f32, tag="ps")
        for u in range(2):
            j = 2 * jj + u
            nc.tensor.matmul(
                pv[:, u, 0:C],
                x8_slice(j * P, P),
                wvo_sb[:, :, :],
                start=True, stop=True, perf_mode=DR)
        nc.scalar.activation(
            v8[:, 2 * jj:2 * jj + 2, 0:C], pv[:, :, 0:C], AF.Copy,
            scale=1.0 / 16.0)

    # ---- density part B (gray -> skv), convs on Pool ----
    avar = _make_row_shifted(nc, const, abs_t, "abs")

    def conv_taps(out_t, in_var, wpat):
        for i, (ky, kx) in enumerate(TAPS):
            dy, dx = ky - 1, kx - 1
            c0, c1 = max(0, -dx), WW - max(0, dx)
            L = c1 - c0
            src = in_var(dy, slice(c0 + dx, c1 + dx))
            w = wpat[:, ky * 3 + kx, :, :].broadcast_to([64, 8, L])
            if i == 0:
                nc.vector.tensor_mul(out_t[:, :, :], src, w)
            else:
                nc.vector.tensor_mul(ctmp_t[:, :, 0:L], src, w)
                nc.vector.tensor_add(
                    out_t[:, :, c0:c1], out_t[:, :, c0:c1], ctmp_t[:, :, 0:L])

    conv_taps(
        h1_t,
        lambda dy, cs: avar[dy][:, :, cs].broadcast_to(
            [64, 8, cs.stop - cs.start]),
        w1b_sb)
    nc.vector.tensor_add(
        h1_t[:, :, :], h1_t[:, :, :], w1bias_sb.broadcast_to([64, 8, WW]))
    nc.vector.tensor_scalar(
        out=h1r_t[:, :, :], in0=h1_t[:, :, :], scalar1=0.0, scalar2=None,
        op0=ALU.max)

    hvar = _make_row_shifted(nc, const, h1r_t, "h1r")
    conv_taps(cacc_t, lambda dy, cs: hvar[dy][:, :, cs], w2b_sb)
    nc.vector.tensor_add(cacc_t[:, 0:4, :], cacc_t[:, 0:4, :], cacc_t[:, 4:8, :])
    nc.vector.tensor_add(cacc_t[:, 0:2, :], cacc_t[:, 0:2, :], cacc_t[:, 2:4, :])
    nc.vector.tensor_add(dl_t[:, :], cacc_t[:, 0, :], cacc_t[:, 1, :])
    nc.scalar.activation(sig_t[:, :], dl_t[:, :], AF.Sigmoid, bias=w2bias_sb[:, 0:1])
    # skv = (C^-0.5/16) / (3 - 2*sigmoid); /16 compensates the 16x k weights
    nc.scalar.activation(dl_t[:, :], sig_t[:, :], AF.Copy, bias=3.0, scale=-2.0)
    nc.vector.reciprocal(sig_t[:, :], dl_t[:, :])
    nc.vector.tensor_scalar(
        out=skv_t[:, :], in0=sig_t[:, :], scalar1=float(C) ** -0.5 / 16.0,
        scalar2=None, op0=ALU.mult)
    nc.sync.dma_start(scr2.rearrange("(h w) -> h w", w=64), skv_t[:, :])

    # skv broadcast [P, N] (chunked so the k8 fold can start early) and the
    # pool fold k8 = k16 * skv
    scr2_1 = scr2.rearrange("(a b) -> a b", a=1)
    for nt in range(NQ_TILES):
        sl = slice(nt * NQT, (nt + 1) * NQT)
        nc.sync.dma_start(
            repl[:, 0, sl], scr2_1[0:1, sl].broadcast_to([P, NQT]))
        nc.vector.tensor_mul(
            k8[:, :, sl], k16[:, :, sl],
            repl[:, :, sl].broadcast_to([P, CC, NQT]))


    # x16b = x + bfin (residual with the final bias folded); ACT, emitted
    # here so the upfront eviction streams aren't delayed behind it
    for t in range(4):
        for ci in range(CC):
            nc.scalar.activation(
                x16b[t][:, ci, :], x_parts[t][:, ci, :].bitcast(f32),
                AF.Identity, bias=bfin_sb[:, ci:ci + 1])

    # ---- attention loop ----
    # scr3 linear index it*512 + s*128 + p must receive rcp[p, s]: view the
    # DRAM as [it, p, s] for the write, [it, q] (q = s*128+p) for the read
    scr3_3 = scr3.rearrange("(it s p) -> it p s", s=NSUB, p=P)
    scr3_2 = scr3.rearrange("(it q) -> it q", q=NQT)
    for it in range(NQ_TILES):
        nq0 = it * NQT
        exp_sb = exp_a if it % 2 == 0 else exp_b

        # q tile projection (fp8 DR), DVE evict straight to e4 with bias
        q8_t = qt_pool.tile([P, CC, NQT], e4)
        pq = ps_pool.tile([P, CC, NQT], f32, tag="ps")
        for mm in range(CC):
            nc.tensor.matmul(
                pq[:, mm, :],
                wqk_sb[:, :, mm * P:(mm + 1) * P],
                x8_slice(nq0, NQT),
                start=True, stop=True, perf_mode=DR)
        for mm in range(CC):
            nc.vector.tensor_scalar(
                out=q8_t[:, mm, :], in0=pq[:, mm, :],
                scalar1=qkbq_sb[:, mm:mm + 1], scalar2=None, op0=ALU.add)

        pos = pos_pool.tile([P, NSUB, NQT], f32)

        def attnv_pair(jj):
            for s in range(NSUB):
                nc.tensor.matmul(
                    pos[:, s, 0:C + 1],
                    exp_sb[:, 2 * jj:2 * jj + 2, s * P:(s + 1) * P],
                    v8[:, 2 * jj:2 * jj + 2, :],
                    start=(jj == 0), stop=(jj == NPAIR - 1), perf_mode=DR)

        for jj in range(NPAIR):
            ps2 = ps_pool.tile([P, 2, NQT], f32, tag="ps")
            for u in range(2):
                j = 2 * jj + u
                nc.tensor.matmul(
                    ps2[:, u, :],
                    k8[:, :, j * P:(j + 1) * P],
                    q8_t[:, :, :],
                    start=True, stop=True, perf_mode=DR)
            dst = exp_sb[:, 2 * jj:2 * jj + 2, :]
            if jj in DVE_JJ:
                nc.vector.tensor_scalar(
                    out=dst.bitcast(i8), in0=ps2[:, :, :],
                    scalar1=SCH_A, scalar2=SCH_B, op0=ALU.mult, op1=ALU.add)
            else:
                nc.scalar.activation(dst, ps2[:, :, :], AF.Exp, scale=SCORE_DESCALE)
            if jj >= 1:
                attnv_pair(jj - 1)
        attnv_pair(NPAIR - 1)

        # rowsum reciprocals -> DRAM roundtrip -> broadcast over partitions
        # (dst AP is the transposed DRAM view so SBUF keeps partition-first)
        rcp = rcp_pool.tile([P, NSUB], f32)
        nc.vector.reciprocal(rcp[:, :], pos[:, :, C])
        nc.sync.dma_start(scr3_3[it, :, :], rcp[:, :])
        rcpb = rcpb_pool.tile([P, 1, NQT], f32)
        nc.sync.dma_start(
            rcpb[:, 0, :], scr3_2[it:it + 1, :].broadcast_to([P, NQT]))

        # ACT evicts attn@V block to bf16 [q, (s), c]
        osbd = osbd_pool.tile([P, NSUB, C], bf16)
        nc.scalar.activation(osbd[:, :, :], pos[:, :, 0:C], AF.Copy)

        # PE transposes each [128q, 128c] block into a shared 2-bank PSUM
        # tile (slot ci*4+s); start once per bank, accumulate onto the
        # pending-zero region for the other slots
        pt = ps_pool.tile([P, CC * NSUB, 2 * P], bf16, tag="ps")
        for ci in range(CC):
            for s in range(NSUB):
                nc.tensor.matmul(
                    pt[:, ci * NSUB + s, 0:P],
                    osbd[:, s, ci * P:(ci + 1) * P], ident16[:, :],
                    is_transpose=True, start=(s == 0), stop=(s == NSUB - 1))

        # DVE evicts pt (freeing the ps buffer), pool normalizes + residual
        t0 = osbT_pool.tile([P, CC, NQT], bf16)
        nc.vector.tensor_copy(t0[:, :, :], pt[:, :, 0:P])
        fin = fin_pool.tile([P, CC, NQT], f32)
        nc.gpsimd.tensor_mul(
            fin[:, :, :], t0[:, :, :],
            rcpb[:, :, :].broadcast_to([P, CC, NQT]))
        nc.gpsimd.tensor_add(
            fin[:, :, :], fin[:, :, :], x16b_slice(nq0, NQT))
        for ci in range(CC):
            for h in range(2):
                nc.sync.dma_start(
                    out_d[ci * P:(ci + 1) * P,
                          nq0 + h * (NQT // 2):nq0 + (h + 1) * (NQT // 2)],
                    fin[:, ci, h * (NQT // 2):(h + 1) * (NQT // 2)])


def build_nc():
    nc = bacc.Bacc("TRN2", target_bir_lowering=False, debug=False)
    d = {}

    def inp(name, shape, dt=f32):
        d[name] = nc.dram_tensor(name, shape, dt, kind="ExternalInput").ap()

    inp("x", (C, N), f32r)
    inp("wqk8", (C, 2 * C), e4)
    inp("wvo8", (C, C), e4)
    inp("ones", (P, 1), f32r)
    inp("qkb16q", (P, CC))
    inp("qkb16k", (P, CC))
    inp("bfin", (P, CC))
    inp("w1b", (64, 72), bf16)
    inp("w1bias", (64, 8), bf16)
    inp("w2b", (64, 72), bf16)
    inp("w2bias", (64, 1))
    d["out"] = nc.dram_tensor("out", (C, N), f32, kind="ExternalOutput").ap()
    d["scr1"] = nc.dram_tensor("scr1", (N,), f32, kind="Internal").ap()
    d["scr2"] = nc.dram_tensor("scr2", (N,), f32, kind="Internal").ap()
    d["scr3"] = nc.dram_tensor("scr3", (N,), f32, kind="Internal").ap()

    with tile.TileContext(nc) as tc, ExitStack() as ctx:
        build_kernel_body(tc, ctx, d)
    nc.compile()
    return nc


def host_inputs(x, qkv_w, qkv_b, out_w, out_b, d1_w, d1_b, d2_w, d2_b):
    f = np.float32
    x = np.asarray(x, f)
    wq = np.asarray(qkv_w, f)[:, :, 0, 0]          # [768, 256]
    qkv_b = np.asarray(qkv_b, f)
    wout = np.asarray(out_w, f)[:, :, 0, 0]        # [256, 256]
    out_b = np.asarray(out_b, f)
    e4m3 = ml_dtypes.float8_e4m3
    shared = {
        "wqk8": np.ascontiguousarray(16.0 * wq[0:2 * C].T).astype(e4m3),
        "wvo8": np.ascontiguousarray(
            16.0 * (wout @ wq[2 * C:3 * C]).T).astype(e4m3),
        "qkb16q": np.ascontiguousarray(
            16.0 * qkv_b[0:C].reshape(CC, P).T.astype(f)),
        "qkb16k": np.ascontiguousarray(
            16.0 * qkv_b[C:2 * C].reshape(CC, P).T.astype(f)),
        "bfin": np.ascontiguousarray(
            (wout @ qkv_b[2 * C:3 * C] + out_b).reshape(CC, P).T.astype(f)),
        "w1b": np.tile(
            np.ascontiguousarray(np.asarray(d1_w, f).reshape(8, 9).T).reshape(1, 72),
            (64, 1)).astype(f),
        "w1bias": np.tile(np.asarray(d1_b, f).reshape(1, 8), (64, 1)).astype(f),
        "w2b": np.tile(
            np.ascontiguousarray(np.asarray(d2_w, f).reshape(8, 9).T).reshape(1, 72),
            (64, 1)).astype(f),
        "w2bias": np.tile(np.asarray(d2_b, f).reshape(1, 1), (64, 1)).astype(f),
        "ones": np.full((P, 1), 1.0 / C, f),
    }
    for kk in ("w1b", "w1bias", "w2b"):
        shared[kk] = shared[kk].astype(ml_dtypes.bfloat16)
    xs = x.reshape(B, C, N)
    return [dict(x=np.ascontiguousarray(xs[b]), **shared) for b in range(B)]


_NC_CACHE = {}


def _get_nc():
    if "nc" not in _NC_CACHE:
        _NC_CACHE["nc"] = build_nc()
    return _NC_CACHE["nc"]


def kernel(x, qkv_w, qkv_b, out_w, out_b, d1_w, d1_b, d2_w, d2_b):
    in_maps = host_inputs(x, qkv_w, qkv_b, out_w, out_b, d1_w, d1_b, d2_w, d2_b)
    nc = _get_nc()
    trace = bool(int(os.environ.get("KERNEL_TRACE", "0")))
    res = bass_utils.run_bass_kernel_spmd(
        nc, in_maps, core_ids=list(range(B)), trace=trace)
    _NC_CACHE["last_results"] = res
    out = np.stack([res.results[b]["out"] for b in range(B)])
    return np.ascontiguousarray(out.reshape(B, C, HH, WW).astype(np.float32))


# revision 19
# speedup vs baseline: 1.2119x; 1.1297x over previous
"""Trainium2 Bass kernel for nn_CDA_attention (density-modulated attention).

Contract: kernel(**full_inputs) -> full output [8, 256, 64, 64] float32.
Data-parallel over batch: core b computes batch b.

v2: fp8 DoubleRow matmuls (0.5 cycles/row, 256-contraction per instruction)
for q/k/v projections, QK^T and attn@V; exp split between ACT (native Exp ->
fp8e5) and DVE (one-pass Schraudolph: int8 convert of a*s+b bitcast as fp8e5);
per-key softmax scale folded into the fp8 k by the Pool engine; final
normalize+transpose+residual via DMA-transpose + Pool with f32 residual.

Layouts per core (batch b, C=256, N=4096):
  x_parts   [P, CC, N] f32r   (residual + gray)
  x8        [P, CC, N] e4m3   (projection operand; pool convert)
  k8        [P, CC, N] e4m3   = (Wk16 x + 16 kb) * skv / 16  (skv = C^-.5/tau)
  q8/it     [P, CC, NQT] e4m3 = Wq16 x + 16 qb   (16x weight scale cancels
                                the 1/16 in k8's fold inside the QK product)
  v8        [P, NKC, C+1] e4m3 = ((Wout Wv) x)/16 transposed, ones column
  exp_sb    [P, NKC, NQT] e5m2 = exp(scores)
  pos       [P, NSUB, 512] f32 psum: attn@V + rowsums (col 256)
  osbd -> dma-transpose -> osbT -> *rcp_bcast + x16b -> out
"""

import os
import sys

sys.path.insert(0, "/opt/trn_rl_repo")

from contextlib import ExitStack

import numpy as np
import ml_dtypes

import concourse.bass as bass
import concourse.mybir as mybir
import concourse.tile as tile
from concourse import bacc, bass_utils
from concourse.masks import make_identity

B, C, HH, WW = 8, 256, 64, 64
N = HH * WW          # 4096
P = 128
CC = C // P          # 2 channel chunks
NQT = 512            # query tile
NQ_TILES = N // NQT  # 8
NKC = N // P         # 32 key chunks
NPAIR = NKC // 2     # 16 key-chunk pairs
NSUB = NQT // P      # 4 query sub-tiles

f32 = mybir.dt.float32
f32r = mybir.dt.float32r
bf16 = mybir.dt.bfloat16
e4 = mybir.dt.float8e4
e5 = mybir.dt.float8e5
i8 = mybir.dt.int8
AF = mybir.ActivationFunctionType
ALU = mybir.AluOpType
PM = mybir.MatmulPerfMode
DR = PM.DoubleRow

LOG2E = 1.4426950408889634
# scores arrive in PSUM as 16x the true scaled score (q carries the 16x host
# weight scale; k's fold divides its own 16x out) -> both exp paths rescale
SCORE_DESCALE = 1.0 / 16.0
SCH_A = 4.0 * LOG2E * SCORE_DESCALE  # e5m2 exponent-domain scale
SCH_B = 60.0          # 15(bias)*4; constant factor cancels in softmax

# which key-chunk pairs the DVE (Schraudolph) path handles; ACT takes the rest
DVE_JJ = frozenset({1, 4, 6, 9, 11, 14})

# tap order for 3x3 convs: center first so the first tap writes the full tile
TAPS = [(1, 1)] + [(ky, kx) for ky in range(3) for kx in range(3) if (ky, kx) != (1, 1)]


def _make_row_shifted(nc, pool, src, name):
    """{dy: AP} of row-shifted copies of src ([64, ...] SBUF tile) via DMA."""
    shape = list(src.shape)
    p1 = pool.tile(shape, src.dtype, name=f"{name}_p1")
    m1 = pool.tile(shape, src.dtype, name=f"{name}_m1")
    nc.gpsimd.memset(p1[:], 0.0)
    nc.gpsimd.memset(m1[:], 0.0)
    nc.sync.dma_start(p1[0:63], src[1:64])
    nc.sync.dma_start(m1[1:64], src[0:63])
    return {0: src, 1: p1, -1: m1}


def build_kernel_body(tc, ctx, d):
    nc = tc.nc
    x_d, wqk_d, wvo_d = d["x"], d["wqk8"], d["wvo8"]
    qkbq_d, qkbk_d, bfin_d = d["qkb16q"], d["qkb16k"], d["bfin"]
    w1b_d, w1bias_d, w2b_d, w2bias_d = d["w1b"], d["w1bias"], d["w2b"], d["w2bias"]
    out_d, scr1, scr2, scr3 = d["out"], d["scr1"], d["scr2"], d["scr3"]

    const = ctx.enter_context(tc.tile_pool(name="const", bufs=1))
    big = ctx.enter_context(tc.tile_pool(name="big", bufs=1))
    ps_pool = ctx.enter_context(tc.tile_pool(name="ps", bufs=2, space="PSUM"))
    pos_pool = ctx.enter_context(tc.tile_pool(name="pos", bufs=1, space="PSUM"))
    qt_pool = ctx.enter_context(tc.tile_pool(name="qt", bufs=2))
    osbd_pool = ctx.enter_context(tc.tile_pool(name="osbd", bufs=2))
    osbT_pool = ctx.enter_context(tc.tile_pool(name="osbT", bufs=2))
    fin_pool = ctx.enter_context(tc.tile_pool(name="fin", bufs=2))
    rcp_pool = ctx.enter_context(tc.tile_pool(name="rcp", bufs=2))
    rcpb_pool = ctx.enter_context(tc.tile_pool(name="rcpb", bufs=2))
    grow_pool = ctx.enter_context(tc.tile_pool(name="grow", bufs=2))

    # ---- persistent SBUF tiles ----
    XQ = N // 4
    x_parts = [big.tile([P, CC, XQ], f32r, name=f"xp{t}") for t in range(4)]

    def x_slice(ci, start, size):
        t = start // XQ
        o = start - t * XQ
        return x_parts[t][:, ci, o:o + size]

    x8 = [big.tile([P, CC, XQ], e4, name=f"x8_{t}") for t in range(4)]

    def x8_slice(start, size):
        t = start // XQ
        o = start - t * XQ
        return x8[t][:, :, o:o + size]

    x16b = [big.tile([P, CC, XQ], f32, name=f"x16b_{t}") for t in range(4)]

    def x16b_slice(start, size):
        t = start // XQ
        o = start - t * XQ
        return x16b[t][:, :, o:o + size]

    k16 = big.tile([P, CC, N], bf16)
    k8 = big.tile([P, CC, N], e4)
    v8 = big.tile([P, NKC, C + 1], e4)
    exp_a = big.tile([P, NKC, NQT], e5)
    exp_b = big.tile([P, NKC, NQT], e5)
    repl = big.tile([P, 1, N], bf16)
    wqk_sb = const.tile([P, CC, 2 * C], e4)
    wvo_sb = const.tile([P, CC, C], e4)
    qkbq_sb = const.tile([P, CC], f32)
    qkbk_sb = const.tile([P, CC], f32)
    bfin_sb = const.tile([P, CC], f32)
    w1b_sb = const.tile([64, 9, 8, 1], bf16)
    w1bias_sb = const.tile([64, 8, 1], bf16)
    w2b_sb = const.tile([64, 9, 8, 1], bf16)
    w2bias_sb = const.tile([64, 1], f32)
    ones_sb = const.tile([P, 1], f32r)
    ident16 = const.tile([P, P], bf16)
    gray_img = const.tile([64, 64], f32)
    lap_t = const.tile([64, 64], f32)
    abs_t = const.tile([64, 1, 64], bf16)
    h1_t = const.tile([64, 8, 64], bf16)
    h1r_t = const.tile([64, 8, 64], bf16)
    cacc_t = const.tile([64, 8, 64], bf16)
    ctmp_t = const.tile([64, 8, 64], bf16)
    dl_t = const.tile([64, 64], bf16)
    sig_t = const.tile([64, 64], f32)
    skv_t = const.tile([64, 64], bf16)

    # ---- input DMAs (x first: it gates everything) ----
    for t in range(4):
        for ci in range(CC):
            for h in range(2):
                nc.sync.dma_start(
                    x_parts[t][:, ci, h * (XQ // 2):(h + 1) * (XQ // 2)],
                    x_d[ci * P:(ci + 1) * P,
                        t * XQ + h * (XQ // 2):t * XQ + (h + 1) * (XQ // 2)])
    for ci in range(CC):
        nc.sync.dma_start(wqk_sb[:, ci, :], wqk_d[ci * P:(ci + 1) * P, :])
        nc.sync.dma_start(wvo_sb[:, ci, :], wvo_d[ci * P:(ci + 1) * P, :])
    nc.sync.dma_start(ones_sb[:, :], d["ones"][:, :])
    nc.sync.dma_start(qkbq_sb[:, :], qkbq_d[:, :])
    nc.sync.dma_start(qkbk_sb[:, :], qkbk_d[:, :])
    nc.sync.dma_start(bfin_sb[:, :], bfin_d[:, :])
    nc.sync.dma_start(w1b_sb[:, :, :, 0], w1b_d.rearrange("p (t o) -> p t o", o=8))
    nc.sync.dma_start(w1bias_sb[:, :, 0], w1bias_d[:, :])
    nc.sync.dma_start(w2b_sb[:, :, :, 0], w2b_d.rearrange("p (t o) -> p t o", o=8))
    nc.sync.dma_start(w2bias_sb[:, :], w2bias_d[:, :])

    nc.gpsimd.memset(v8[:, :, C:C + 1], 1.0)   # ones columns -> row sums
    make_identity(nc, ident16)

    # fp8 x copy (projection operand), per quarter as DMA lands; split between
    # ACT and DVE (both idle during the input DMA window; Pool is too slow)
    for t in range(4):
        if t < 2:
            nc.scalar.activation(
                x8[t][:, :, :], x_parts[t][:, :, :].bitcast(f32), AF.Copy)
        else:
            nc.vector.tensor_copy(x8[t][:, :, :], x_parts[t][:, :, :].bitcast(f32))


    # ---- gray = mean_c x (PE, f32r) ----
    scr1_2d = scr1.rearrange("(a b) -> a b", a=1)
    for nt in range(NQ_TILES):
        pg = ps_pool.tile([1, NQT], f32, tag="ps")
        for ci in range(CC):
            nc.tensor.matmul(
                pg[:, :], ones_sb[:, :], x_slice(ci, nt * NQT, NQT),
                start=(ci == 0), stop=(ci == CC - 1))
        grow = grow_pool.tile([1, NQT], f32)
        nc.vector.tensor_copy(grow[:, :], pg[:, :])
        nc.sync.dma_start(scr1_2d[:, nt * NQT:(nt + 1) * NQT], grow[:, :])

    nc.sync.dma_start(gray_img[:, :], scr1.rearrange("(h w) -> h w", w=64))
    g_p1 = const.tile([64, 64], f32)
    g_m1 = const.tile([64, 64], f32)
    nc.gpsimd.memset(g_p1[:], 0.0)
    nc.gpsimd.memset(g_m1[:], 0.0)
    sh = scr1.rearrange("(h w) -> h w", w=64)
    nc.sync.dma_start(g_p1[0:63, :], sh[1:64, :])
    nc.sync.dma_start(g_m1[1:64, :], sh[0:63, :])
    gvar = {0: gray_img, 1: g_p1, -1: g_m1}

    # ---- density part A: Laplacian (DVE) + abs (ACT), emitted BEFORE the
    # projections so these small ops sit ahead of the eviction streams in the
    # in-order DVE/ACT queues ----
    nc.vector.tensor_scalar(
        out=lap_t[:, :], in0=gray_img[:, :], scalar1=4.0, scalar2=None, op0=ALU.mult)
    for dy in (1, -1):
        nc.vector.scalar_tensor_tensor(
            out=lap_t[:, :], in0=gvar[dy][:, :], scalar=-1.0, in1=lap_t[:, :],
            op0=ALU.mult, op1=ALU.add)
    for dx in (1, -1):
        c0, c1 = max(0, -dx), WW - max(0, dx)
        dst = lap_t[:, c0:c1]
        nc.vector.scalar_tensor_tensor(
            out=dst, in0=gray_img[:, c0 + dx:c1 + dx], scalar=-1.0, in1=dst,
            op0=ALU.mult, op1=ALU.add)
    nc.scalar.activation(abs_t[:, 0, :], lap_t[:, :], AF.Abs)

    # ---- k projection (fp8 DoubleRow) + DVE evict to bf16 ----
    for nt in range(NQ_TILES):
        pq = ps_pool.tile([P, CC, NQT], f32, tag="ps")
        for m in range(CC):                # k output-channel chunks
            nc.tensor.matmul(
                pq[:, m, :],
                wqk_sb[:, :, (2 + m) * P:(3 + m) * P],
                x8_slice(nt * NQT, NQT),
                start=True, stop=True, perf_mode=DR)
        for m in range(CC):
            nc.scalar.activation(
                k16[:, m, nt * NQT:(nt + 1) * NQT], pq[:, m, :],
                AF.Identity, bias=qkbk_sb[:, m:m + 1])

    # ---- vproj = ((Wout Wv)/16) x transposed (fp8 DR) + ACT evict ----
    for jj in range(NPAIR):
        pv = ps_pool.tile([P, 2, NQT], f32, tag="ps")
        for u in range(2):
            j = 2 * jj + u
            nc.tensor.matmul(
                pv[:, u, 0:C],
                x8_slice(j * P, P),
                wvo_sb[:, :, :],
                start=True, stop=True, perf_mode=DR)
        nc.scalar.activation(
            v8[:, 2 * jj:2 * jj + 2, 0:C], pv[:, :, 0:C], AF.Copy,
            scale=1.0 / 16.0)

    # ---- density part B (gray -> skv), convs on Pool ----
    avar = _make_row_shifted(nc, const, abs_t, "abs")

    def conv_taps(out_t, in_var, wpat):
        for i, (ky, kx) in enumerate(TAPS):
            dy, dx = ky - 1, kx - 1
            c0, c1 = max(0, -dx), WW - max(0, dx)
            L = c1 - c0
            src = in_var(dy, slice(c0 + dx, c1 + dx))
            w = wpat[:, ky * 3 + kx, :, :].broadcast_to([64, 8, L])
            if i == 0:
                nc.vector.tensor_mul(out_t[:, :, :], src, w)
            else:
                nc.vector.tensor_mul(ctmp_t[:, :, 0:L], src, w)
                nc.vector.tensor_add(
                    out_t[:, :, c0:c1], out_t[:, :, c0:c1], ctmp_t[:, :, 0:L])

    conv_taps(
        h1_t,
        lambda dy, cs: avar[dy][:, :, cs].broadcast_to(
            [64, 8, cs.stop - cs.start]),
        w1b_sb)
    nc.vector.tensor_add(
        h1_t[:, :, :], h1_t[:, :, :], w1bias_sb.broadcast_to([64, 8, WW]))
    nc.vector.tensor_scalar(
        out=h1r_t[:, :, :], in0=h1_t[:, :, :], scalar1=0.0, scalar2=None,
        op0=ALU.max)

    hvar = _make_row_shifted(nc, const, h1r_t, "h1r")
    conv_taps(cacc_t, lambda dy, cs: hvar[dy][:, :, cs], w2b_sb)
    nc.vector.tensor_add(cacc_t[:, 0:4, :], cacc_t[:, 0:4, :], cacc_t[:, 4:8, :])
    nc.vector.tensor_add(cacc_t[:, 0:2, :], cacc_t[:, 0:2, :], cacc_t[:, 2:4, :])
    nc.vector.tensor_add(dl_t[:, :], cacc_t[:, 0, :], cacc_t[:, 1, :])
    nc.scalar.activation(sig_t[:, :], dl_t[:, :], AF.Sigmoid, bias=w2bias_sb[:, 0:1])
    # skv = (C^-0.5/16) / (3 - 2*sigmoid); /16 compensates the 16x k weights
    nc.scalar.activation(dl_t[:, :], sig_t[:, :], AF.Copy, bias=3.0, scale=-2.0)
    nc.vector.reciprocal(sig_t[:, :], dl_t[:, :])
    nc.vector.tensor_scalar(
        out=skv_t[:, :], in0=sig_t[:, :], scalar1=float(C) ** -0.5 / 16.0,
        scalar2=None, op0=ALU.mult)
    nc.sync.dma_start(scr2.rearrange("(h w) -> h w", w=64), skv_t[:, :])

    # skv broadcast [P, N] (chunked so the k8 fold can start early) and the
    # pool fold k8 = k16 * skv
    scr2_1 = scr2.rearrange("(a b) -> a b", a=1)
    for nt in range(NQ_TILES):
        sl = slice(nt * NQT, (nt + 1) * NQT)
        nc.sync.dma_start(
            repl[:, 0, sl], scr2_1[0:1, sl].broadcast_to([P, NQT]))
        nc.vector.tensor_mul(
            k8[:, :, sl], k16[:, :, sl],
            repl[:, :, sl].broadcast_to([P, CC, NQT]))


    # x16b = x + bfin (residual with the final bias folded); ACT, emitted
    # here so the upfront eviction streams aren't delayed behind it
    for t in range(4):
        for ci in range(CC):
            nc.scalar.activation(
                x16b[t][:, ci, :], x_parts[t][:, ci, :].bitcast(f32),
                AF.Identity, bias=bfin_sb[:, ci:ci + 1])

    # ---- attention loop ----
    # scr3 linear index it*512 + s*128 + p must receive rcp[p, s]: view the
    # DRAM as [it, p, s] for the write, [it, q] (q = s*128+p) for the read
    scr3_3 = scr3.rearrange("(it s p) -> it p s", s=NSUB, p=P)
    scr3_2 = scr3.rearrange("(it q) -> it q", q=NQT)
    for it in range(NQ_TILES):
        nq0 = it * NQT
        exp_sb = exp_a if it % 2 == 0 else exp_b

        # q tile projection (fp8 DR), DVE evict straight to e4 with bias
        q8_t = qt_pool.tile([P, CC, NQT], e4)
        pq = ps_pool.tile([P, CC, NQT], f32, tag="ps")
        for mm in range(CC):
            nc.tensor.matmul(
                pq[:, mm, :],
                wqk_sb[:, :, mm * P:(mm + 1) * P],
                x8_slice(nq0, NQT),
                start=True, stop=True, perf_mode=DR)
        for mm in range(CC):
            nc.vector.tensor_scalar(
                out=q8_t[:, mm, :], in0=pq[:, mm, :],
                scalar1=qkbq_sb[:, mm:mm + 1], scalar2=None, op0=ALU.add)

        pos = pos_pool.tile([P, NSUB, NQT], f32, tag="pos")

        def attnv_pair(jj):
            for s in range(NSUB):
                nc.tensor.matmul(
                    pos[:, s, 0:C + 1],
                    exp_sb[:, 2 * jj:2 * jj + 2, s * P:(s + 1) * P],
                    v8[:, 2 * jj:2 * jj + 2, :],
                    start=(jj == 0), stop=(jj == NPAIR - 1), perf_mode=DR)

        for jj in range(NPAIR):
            ps2 = ps_pool.tile([P, 2, NQT], f32, tag="ps")
            for u in range(2):
                j = 2 * jj + u
                nc.tensor.matmul(
                    ps2[:, u, :],
                    k8[:, :, j * P:(j + 1) * P],
                    q8_t[:, :, :],
                    start=True, stop=True, perf_mode=DR)
            dst = exp_sb[:, 2 * jj:2 * jj + 2, :]
            if jj in DVE_JJ:
                nc.vector.tensor_scalar(
                    out=dst.bitcast(i8), in0=ps2[:, :, :],
                    scalar1=SCH_A, scalar2=SCH_B, op0=ALU.mult, op1=ALU.add)
            else:
                nc.scalar.activation(dst, ps2[:, :, :], AF.Exp, scale=SCORE_DESCALE)
            if jj >= 1:
                attnv_pair(jj - 1)
        attnv_pair(NPAIR - 1)

        # rowsum reciprocals -> DRAM roundtrip -> broadcast over partitions
        # (dst AP is the transposed DRAM view so SBUF keeps partition-first)
        rcp = rcp_pool.tile([P, NSUB], f32)
        nc.vector.reciprocal(rcp[:, :], pos[:, :, C])
        nc.sync.dma_start(scr3_3[it, :, :], rcp[:, :])
        rcpb = rcpb_pool.tile([P, 1, NQT], f32)
        nc.sync.dma_start(
            rcpb[:, 0, :], scr3_2[it:it + 1, :].broadcast_to([P, NQT]))

        # ACT evicts attn@V block to bf16 [q, (s), c]
        osbd = osbd_pool.tile([P, NSUB, C], bf16)
        nc.scalar.activation(osbd[:, :, :], pos[:, :, 0:C], AF.Copy)

        # PE transposes each [128q, 128c] block into a shared 2-bank PSUM
        # tile (slot ci*4+s); start once per bank, accumulate onto the
        # pending-zero region for the other slots
        pt = pos_pool.tile([P, CC * NSUB, 2 * P], bf16, tag="pos")
        for ci in range(CC):
            for s in range(NSUB):
                nc.tensor.matmul(
                    pt[:, ci * NSUB + s, 0:P],
                    osbd[:, s, ci * P:(ci + 1) * P], ident16[:, :],
                    is_transpose=True, start=(s == 0), stop=(s == NSUB - 1))

        # DVE evicts pt (freeing the ps buffer), pool normalizes + residual
        t0 = osbT_pool.tile([P, CC, NQT], bf16)
        nc.vector.tensor_copy(t0[:, :, :], pt[:, :, 0:P])
        fin = fin_pool.tile([P, CC, NQT], f32)
        nc.gpsimd.tensor_mul(
            fin[:, :, :], t0[:, :, :],
            rcpb[:, :, :].broadcast_to([P, CC, NQT]))
        nc.gpsimd.tensor_add(
            fin[:, :, :], fin[:, :, :], x16b_slice(nq0, NQT))
        for ci in range(CC):
            for h in range(2):
                nc.sync.dma_start(
                    out_d[ci * P:(ci + 1) * P,
                          nq0 + h * (NQT // 2):nq0 + (h + 1) * (NQT // 2)],
                    fin[:, ci, h * (NQT // 2):(h + 1) * (NQT // 2)])


def build_nc():
    nc = bacc.Bacc("TRN2", target_bir_lowering=False, debug=False)
    d = {}

    def inp(name, shape, dt=f32):
        d[name] = nc.dram_tensor(name, shape, dt, kind="ExternalInput").ap()

    inp("x", (C, N), f32r)
    inp("wqk8", (C, 2 * C), e4)
    inp("wvo8", (C, C), e4)
    inp("ones", (P, 1), f32r)
    inp("qkb16q", (P, CC))
    inp("qkb16k", (P, CC))
    inp("bfin", (P, CC))
    inp("w1b", (64, 72), bf16)
    inp("w1bias", (64, 8), bf16)
    inp("w2b", (64, 72), bf16)
    inp("w2bias", (64, 1))
    d["out"] = nc.dram_tensor("out", (C, N), f32, kind="ExternalOutput").ap()
    d["scr1"] = nc.dram_tensor("scr1", (N,), f32, kind="Internal").ap()
    d["scr2"] = nc.dram_tensor("scr2", (N,), bf16, kind="Internal").ap()
    d["scr3"] = nc.dram_tensor("scr3", (N,), f32, kind="Internal").ap()

    with tile.TileContext(nc) as tc, ExitStack() as ctx:
        build_kernel_body(tc, ctx, d)
    nc.compile()
    return nc


def host_inputs(x, qkv_w, qkv_b, out_w, out_b, d1_w, d1_b, d2_w, d2_b):
    f = np.float32
    x = np.asarray(x, f)
    wq = np.asarray(qkv_w, f)[:, :, 0, 0]          # [768, 256]
    qkv_b = np.asarray(qkv_b, f)
    wout = np.asarray(out_w, f)[:, :, 0, 0]        # [256, 256]
    out_b = np.asarray(out_b, f)
    e4m3 = ml_dtypes.float8_e4m3
    shared = {
        "wqk8": np.ascontiguousarray(16.0 * wq[0:2 * C].T).astype(e4m3),
        "wvo8": np.ascontiguousarray(
            16.0 * (wout @ wq[2 * C:3 * C]).T).astype(e4m3),
        "qkb16q": np.ascontiguousarray(
            16.0 * qkv_b[0:C].reshape(CC, P).T.astype(f)),
        "qkb16k": np.ascontiguousarray(
            16.0 * qkv_b[C:2 * C].reshape(CC, P).T.astype(f)),
        "bfin": np.ascontiguousarray(
            (wout @ qkv_b[2 * C:3 * C] + out_b).reshape(CC, P).T.astype(f)),
        "w1b": np.tile(
            np.ascontiguousarray(np.asarray(d1_w, f).reshape(8, 9).T).reshape(1, 72),
            (64, 1)).astype(f),
        "w1bias": np.tile(np.asarray(d1_b, f).reshape(1, 8), (64, 1)).astype(f),
        "w2b": np.tile(
            np.ascontiguousarray(np.asarray(d2_w, f).reshape(8, 9).T).reshape(1, 72),
            (64, 1)).astype(f),
        "w2bias": np.tile(np.asarray(d2_b, f).reshape(1, 1), (64, 1)).astype(f),
        "ones": np.full((P, 1), 1.0 / C, f),
    }
    for kk in ("w1b", "w1bias", "w2b"):
        shared[kk] = shared[kk].astype(ml_dtypes.bfloat16)
    xs = x.reshape(B, C, N)
    return [dict(x=np.ascontiguousarray(xs[b]), **shared) for b in range(B)]


_NC_CACHE = {}


def _get_nc():
    if "nc" not in _NC_CACHE:
        _NC_CACHE["nc"] = build_nc()
    return _NC_CACHE["nc"]


def kernel(x, qkv_w, qkv_b, out_w, out_b, d1_w, d1_b, d2_w, d2_b):
    in_maps = host_inputs(x, qkv_w, qkv_b, out_w, out_b, d1_w, d1_b, d2_w, d2_b)
    nc = _get_nc()
    trace = bool(int(os.environ.get("KERNEL_TRACE", "0")))
    res = bass_utils.run_bass_kernel_spmd(
        nc, in_maps, core_ids=list(range(B)), trace=trace)
    _NC_CACHE["last_results"] = res
    out = np.stack([res.results[b]["out"] for b in range(B)])
    return np.ascontiguousarray(out.reshape(B, C, HH, WW).astype(np.float32))
